# revision 7
# baseline (speedup 1.0000x reference)
"""Multi-head attention Bass kernel for Trainium2, sharded over 8 NeuronCores.

Problem: x [2, 2048, 1024] fp32; W_qkv [3072, 1024]; b_qkv [3072].
  qkv = x @ W_qkv.T + b_qkv ; split into Q,K,V of 8 heads x 128 dims;
  out  = softmax(Q K^T / sqrt(128)) V, heads re-concatenated -> [2, 2048, 1024].

Sharding: 16 (batch, head) pairs over 8 cores -> each core owns one batch
slice (b = core//4) and two heads (h0 = 2*(core%4), h0+1). No collectives.

v3 (fp8 scores + dual-ring DMA + planner-paced emission):
 - x streams over BOTH HWDGE rings (sync + scalar) in interleaved ko chunks,
   so the full 4MB lands in ~7us instead of ~17us and phase 0 is never
   DMA-gated past the first chunk (the PE p-state ramps to 2.4GHz and stays).
 - Phase 0 computes K0 only (4 PSUM banks, ko-outer chunk-gated), then Q0's
   first token block, so the first score matmul and the ACT exp stream start
   ~5us earlier than v2's combined K0/Q0 phase.
 - Scores run in fp8e4m3 with MatmulPerfMode.DoubleRow at 2x rate: Q/K
   drains write fp8 directly [128, tensor, tok]; small SBUF->SBUF DMAs on
   the sync ring shuffle them into the [64, 2, tok] two-subtile layout the
   DoubleRow contraction needs (dh split 0:64 / 64:128).  P*V stays bf16
   (fp8 there would blow the 2e-2 error budget; measured L2 ~1.6e-2).
 - Instruction emission is paced by a small host-side timeline planner that
   models PE/ACT progress and orders score groups, PV chunks, and projection
   fillers (Q0/K1/Q1/V0/V1) so the in-order engines never sit on a gated
   instruction while other ready work exists.  ACT (the exp stream, ~69us)
   stays saturated; the PE (~82us of matmul) is the critical engine.
 - Same math as v2 otherwise: scores transposed (S^T tiles [k, q]), exp on
   ACT with the 1/sqrt(dh) scale folded in (no max subtraction; scores are
   O(1)), PV with a ones column appended to V so softmax denominators fall
   out of the same matmuls, final scale by 1/denom on DVE.
"""

import math
from contextlib import ExitStack

import numpy as np

import concourse.bass as bass
import concourse.tile as tile
from concourse import bacc, mybir
from concourse.bass_utils import run_bass_kernel_spmd

# Problem constants (hardcoded per the harness contract).
B = 2
S = 2048
D = 1024
H = 8
DH = 128
N_CORES = 8
HPC = 2  # heads per core
SC = S  # tokens per core (one full batch element)
SCALE = 1.0 / math.sqrt(DH)

F32 = mybir.dt.float32
BF16 = mybir.dt.bfloat16
F8 = mybir.dt.float8e4

KO = D // 128  # 8 contraction chunks
QB = 256  # query block width
NQB = SC // QB  # 8
NKT = S // 128  # 16 key tiles
KPS = 4  # key tiles per score/exp group (exp on [128, KPS*QB] = [128,1024])
NG = NKT // KPS  # 4 score groups per query block
NTB = 4  # 512-token tiles for Q/K projection
TB = SC // NTB  # 512

USE_FP8_SCORES = False

# tensor indices in the packed q/k store
TK0, TQ0, TK1, TQ1 = 0, 1, 2, 3
# bias column (in bqk host layout [q0 q1 k0 k1]) for each tensor index
BIAS_COL = {TK0: 2, TQ0: 0, TK1: 3, TQ1: 1}


def _mha_body(ctx: ExitStack, tc: tile.TileContext, out, x, wtak, wtaq, wtb, bqk_d, bias_v):
    nc = tc.nc

    consts = ctx.enter_context(tc.tile_pool(name="consts", bufs=1))
    xtp = ctx.enter_context(tc.tile_pool(name="xtp", bufs=1))
    qkvp = ctx.enter_context(tc.tile_pool(name="qkvp", bufs=1))

    # hoist the ACT exp table load (~2.7us) to kernel start, under the input DMA
    warm = consts.tile([128, 1], F32)
    nc.vector.memset(warm, 0.0)
    nc.scalar.activation(warm, warm, mybir.ActivationFunctionType.Exp)

    # ---- input DMAs: x interleaved over BOTH HWDGE rings ----
    # sync ring:   ko0 (two half-token chunks), ko2, ko4, ko6, wtb
    # scalar ring: wtak, wtaq, bqk, ko1, ko3, ko5, ko7
    # gpsimd SWDGE: broadcast V bias
    xt = xtp.tile([128, KO, SC], BF16)  # [ki, ko, tok]
    nc.sync.dma_start(xt[:, 0:1, 0:SC // 2], x[:, 0:1, 0:SC // 2])
    nc.sync.dma_start(xt[:, 0:1, SC // 2:], x[:, 0:1, SC // 2:])
    wtak_sb = consts.tile([128, KO, DH], BF16)  # [ki, ko, m]: k0 cols
    nc.scalar.dma_start(wtak_sb, wtak)
    wtaq_sb = consts.tile([128, KO, DH], BF16)  # q0 cols
    nc.scalar.dma_start(wtaq_sb, wtaq)
    bqk_sb = consts.tile([128, 2 * HPC], F32)
    nc.scalar.dma_start(bqk_sb, bqk_d)
    wtb_sb = consts.tile([128, KO, 4 * DH], BF16)  # m = [k1 q1 v0 v1]
    for ks, kc in ((2, "sync"), (1, "scalar"), (4, "sync"), (3, "scalar"),
                   (6, "sync"), (5, "scalar"), (7, "scalar")):
        eng = nc.sync if kc == "sync" else nc.scalar
        eng.dma_start(xt[:, ks:ks + 1, :], x[:, ks:ks + 1, :])
    nc.sync.dma_start(wtb_sb, wtb)

    bqk = [bqk_sb[:, i:i + 1] for i in range(2 * HPC)]
    bv_rep = consts.tile([128, HPC * DH], F32)
    nc.gpsimd.dma_start(bv_rep, bias_v[None, :].to_broadcast([128, HPC * DH]))

    # ---- persistent QKV stores ----
    if USE_FP8_SCORES:
        # drains write fp8 here; sync-ring SBUF->SBUF DMAs shuffle into kq8s
        kq8 = qkvp.tile([128, 4, SC], F8, tag="kq8")  # [dh, tensor, tok]
        kq8s = qkvp.tile([64, 2, 4, SC], F8, tag="kq8s")  # [dh%64, dh//64, tensor, tok]
    else:
        kq16 = qkvp.tile([128, 4, SC], BF16, tag="kq16")
    v_sb = qkvp.tile([128, HPC, SC // 128, DH + 1], BF16, tag="v")  # [tok_i, h, tok_o, dh+1]
    nc.vector.memset(v_sb[:, :, :, DH:DH + 1], 1.0)

    def drain_kq(t, tok0, ps, eng):
        b = bqk[BIAS_COL[t]]
        dst = (kq8 if USE_FP8_SCORES else kq16)[:, t, tok0:tok0 + TB]
        if eng == 0:
            nc.vector.tensor_scalar_add(dst, ps, b)
        else:
            nc.scalar.add(dst, ps, b)

    def emit_shuffle(t, tok0, ntok):
        if not USE_FP8_SCORES:
            return
        nc.sync.dma_start(kq8s[:, 0, t, tok0:tok0 + ntok], kq8[0:64, t, tok0:tok0 + ntok])
        nc.sync.dma_start(kq8s[:, 1, t, tok0:tok0 + ntok], kq8[64:128, t, tok0:tok0 + ntok])

    # ---- phase 0: K0 projection ko-outer (chunk-gated), then Q0 tb0 ----
    with ExitStack() as ctx0:
        p0 = ctx0.enter_context(tc.tile_pool(name="p0ps", bufs=1, space="PSUM"))
        k0t = [p0.tile([128, TB], F32, tag=f"k0_{tb}", name=f"k0_{tb}") for tb in range(NTB)]
        q0a = p0.tile([128, TB], F32, tag="q0a", name="q0a")

        def p0_mm(tb, ko, stop=False):
            nc.tensor.matmul(
                k0t[tb],
                lhsT=wtak_sb[:, ko, :],
                rhs=xt[:, ko, tb * TB:(tb + 1) * TB],
                start=(ko == 0),
                stop=stop,
            )

        for ko in range(KO - 2):
            for tb in range(NTB):
                p0_mm(tb, ko)
        # last ko pair tile-major with drains chasing (alternate DVE/ACT)
        for tb in range(NTB):
            p0_mm(tb, KO - 2)
            p0_mm(tb, KO - 1, stop=True)
            drain_kq(TK0, tb * TB, k0t[tb], eng=tb % 2)
        emit_shuffle(TK0, 0, SC)
        # Q0 first token block: unlocks scores(h0, qb0/qb1)
        for ko in range(KO):
            nc.tensor.matmul(
                q0a,
                lhsT=wtaq_sb[:, ko, :],
                rhs=xt[:, ko, 0:TB],
                start=(ko == 0),
                stop=(ko == KO - 1),
            )
        drain_kq(TQ0, 0, q0a, eng=1)
        emit_shuffle(TQ0, 0, TB)

    # ---- main pools (reuse phase-0 PSUM banks; Tile inserts WAR syncs) ----
    st_ps = ctx.enter_context(tc.tile_pool(name="st_ps", bufs=2, space="PSUM"))
    pv_ps = ctx.enter_context(tc.tile_pool(name="pv_ps", bufs=2, space="PSUM"))
    proj_ps = ctx.enter_context(tc.tile_pool(name="proj_ps", bufs=2, space="PSUM"))
    atp = ctx.enter_context(tc.tile_pool(name="atp", bufs=26))
    outp = ctx.enter_context(tc.tile_pool(name="outp", bufs=2))
    rcp = ctx.enter_context(tc.tile_pool(name="rcp", bufs=8))

    def emit_score_group(h, qb, g):
        st = st_ps.tile([128, KPS, QB], F32, tag="st", name="st")
        tk, tq = (TK0, TQ0) if h == 0 else (TK1, TQ1)
        for i in range(KPS):
            kt = g * KPS + i
            if USE_FP8_SCORES:
                nc.tensor.matmul(
                    st[:, i, :],
                    lhsT=kq8s[:, :, tk, kt * 128:(kt + 1) * 128],
                    rhs=kq8s[:, :, tq, qb * QB:(qb + 1) * QB],
                    start=True,
                    stop=True,
                    perf_mode=mybir.MatmulPerfMode.DoubleRow,
                )
            else:
                nc.tensor.matmul(
                    st[:, i, :],
                    lhsT=kq16[:, tk, kt * 128:(kt + 1) * 128],
                    rhs=kq16[:, tq, qb * QB:(qb + 1) * QB],
                    start=True,
                    stop=True,
                )
        at = atp.tile([128, KPS, QB], BF16, tag="at", name="at")
        nc.scalar.activation(at, st, mybir.ActivationFunctionType.Exp, scale=SCALE)
        return at

    def start_pv():
        # separate tiles per j: each PSUM accumulation group must own its own
        # 2KB zero region (a start in a shared region wipes the sibling's data)
        return [pv_ps.tile([128, DH + 1], F32, tag="pv", name=f"pv{j}") for j in range(2)]

    def emit_pv_chunk(h, c, at, pvt):
        for i in range(KPS):
            kt = c * KPS + i
            for j in range(2):
                nc.tensor.matmul(
                    pvt[j],
                    lhsT=at[:, i, j * 128:(j + 1) * 128],
                    rhs=v_sb[:, h, kt, :],
                    start=(kt == 0),
                    stop=(kt == NKT - 1),
                )

    def finish_pv(h, qb, pvt):
        ot = outp.tile([128, 2, DH], F32, tag="ot", name="ot")
        for j in range(2):
            rc = rcp.tile([128, 1], F32, tag="rc", name="rc")
            nc.vector.reciprocal(rc, pvt[j][:, DH:DH + 1])
            nc.vector.tensor_scalar_mul(ot[:, j, :], pvt[j][:, 0:DH], rc)
            nc.sync.dma_start(
                out[qb * QB + j * 128:qb * QB + (j + 1) * 128, h * DH:(h + 1) * DH],
                ot[:, j, :],
            )

    # ---- filler units (projections interleaved into the attention phase) ----
    def proj_unit(t, tb):
        # K1/Q1 (from wtb m-cols 0/1) or Q0 tb>0 (from wtaq)
        ps = proj_ps.tile([128, TB], F32, tag="ps", name="ps")
        for ko in range(KO):
            if t == TQ0:
                w = wtaq_sb[:, ko, :]
            else:
                m = 0 if t == TK1 else 1
                w = wtb_sb[:, ko, m * DH:(m + 1) * DH]
            nc.tensor.matmul(
                ps,
                lhsT=w,
                rhs=xt[:, ko, tb * TB:(tb + 1) * TB],
                start=(ko == 0),
                stop=(ko == KO - 1),
            )
        drain_kq(t, tb * TB, ps, eng=0)
        emit_shuffle(t, tb * TB, TB)

    def v_unit(h, g):
        # V projection for head h, one 128-token tile
        ps = proj_ps.tile([128, TB], F32, tag="ps", name="ps")
        psv = ps[:, :DH]
        for ko in range(KO):
            nc.tensor.matmul(
                psv,
                lhsT=xt[:, ko, g * 128:(g + 1) * 128],
                rhs=wtb_sb[:, ko, (2 + h) * DH:(3 + h) * DH],
                start=(ko == 0),
                stop=(ko == KO - 1),
            )
        nc.vector.tensor_add(v_sb[:, h, g, 0:DH], psv, bv_rep[:, h * DH:(h + 1) * DH])

    # ---- planner-paced emission of the attention phase ----
    # Filler priority order (deps: scores(0,qb) needs Q0 tb qb//2; scores(1,*)
    # need K1 complete + Q1 tb qb//2; PV(h,*,c) needs v units (h, 4c..4c+3)).
    fillers = []
    fillers += [("q0", tb, lambda tb=tb: proj_unit(TQ0, tb), 1.75) for tb in (1, 2, 3)]
    fillers += [("v0", g, lambda g=g: v_unit(0, g), 0.50) for g in range(16)]
    fillers += [("k1", tb, lambda tb=tb: proj_unit(TK1, tb), 1.75) for tb in range(4)]
    fillers += [("q1", 0, lambda: proj_unit(TQ1, 0), 1.75)]
    fillers += [("v1", g, lambda g=g: v_unit(1, g), 0.50) for g in range(8)]
    fillers += [("q1", 1, lambda: proj_unit(TQ1, 1), 1.75)]
    fillers += [("v1", g, lambda g=g: v_unit(1, g), 0.50) for g in range(8, 16)]
    fillers += [("q1", tb, lambda tb=tb: proj_unit(TQ1, tb), 1.75) for tb in (2, 3)]

    SCORE_US = 0.30 if USE_FP8_SCORES else 0.95
    EXP_US = 1.10
    PV_US = 0.50

    done = {("q0", 0): 0.0}  # filler completion times (q0 tb0 done in phase 0)
    order = [(h, qb) for h in range(HPC) for qb in range(NQB)]

    # simulated engine clocks (us, relative to end of phase 0)
    pe_t = 0.0
    act_t = 0.5  # ACT still finishing the q0a drain
    exp_done: dict[int, float] = {}
    at_tiles: dict[int, object] = {}

    si = 0  # next score group index (flat over order x NG)
    pv_qb = 0  # next qb (flat index) whose PV chunks are being emitted
    pv_c = 0
    fi = 0
    pvt_cur = None
    n_slots = len(order) * NG

    def score_ready(flat):
        h, qb = order[flat // NG]
        if h == 0:
            need = [("q0", tb) for tb in range(qb // 2 + 1)]
        else:
            need = [("k1", tb) for tb in range(4)]
            need += [("q1", tb) for tb in range(qb // 2 + 1)]
        if not all(k in done for k in need):
            return None
        t = max([0.0] + [done[k] for k in need])
        # st pool bufs=2: group flat-2's exp must have retired
        if flat >= 2:
            t = max(t, exp_done[flat - 2])
        return t

    def pv_ready(flat, c):
        h, qb = order[flat]
        gflat = flat * NG + c
        if gflat not in exp_done:
            return None
        need = [("v0" if h == 0 else "v1", g) for g in range(c * KPS, (c + 1) * KPS)]
        if not all(k in done for k in need):
            return None
        return max([exp_done[gflat]] + [done[k] for k in need])

    while si < n_slots or pv_qb < len(order):
        # how far ahead is ACT's queued work? feed it if it may starve
        act_lead = act_t - pe_t
        did = None
        if si < n_slots and act_lead < 1.2:
            t = score_ready(si)
            if t is not None and t <= pe_t + 0.05:
                did = "score"
        if did is None and pv_qb < len(order):
            t = pv_ready(pv_qb, pv_c)
            if t is not None and t <= pe_t:
                did = "pv"
        if did is None and si < n_slots:
            t = score_ready(si)
            if t is not None and t <= pe_t + 0.05:
                did = "score"
        if did is None and fi < len(fillers):
            did = "filler"
        if did is None:
            # stall: advance to earliest unblock
            cands = []
            if si < n_slots:
                t = score_ready(si)
                if t is not None:
                    cands.append((t, "score"))
            if pv_qb < len(order):
                t = pv_ready(pv_qb, pv_c)
                if t is not None:
                    cands.append((t, "pv"))
            assert cands, "planner deadlock"
            t, did = min(cands)
            pe_t = max(pe_t, t)

        if did == "score":
            h, qb = order[si // NG]
            g = si % NG
            at_tile = emit_score_group(h, qb, g)
            pe_t += SCORE_US
            act_t = max(act_t, pe_t) + EXP_US
            exp_done[si] = act_t
            at_tiles[si] = at_tile
            si += 1
        elif did == "pv":
            h, qb = order[pv_qb]
            if pv_c == 0:
                pvt_cur = start_pv()
            emit_pv_chunk(h, pv_c, at_tiles.pop(pv_qb * NG + pv_c), pvt_cur)
            pe_t += PV_US
            pv_c += 1
            if pv_c == NG:
                finish_pv(h, qb, pvt_cur)
                pv_c = 0
                pv_qb += 1
        else:  # filler
            kind, idx, fn, cost = fillers[fi]
            fn()
            pe_t += cost
            done[(kind, idx)] = pe_t
            fi += 1

    # any leftover fillers (shouldn't happen, but keep correctness)
    while fi < len(fillers):
        _, _, fn, _ = fillers[fi]
        fn()
        fi += 1


def build_program():
    nc = bacc.Bacc("TRN2", target_bir_lowering=False, debug=False)
    x = nc.dram_tensor("x", [128, KO, SC], BF16, kind="ExternalInput").ap()
    wtak = nc.dram_tensor("wtak", [128, KO, DH], BF16, kind="ExternalInput").ap()
    wtaq = nc.dram_tensor("wtaq", [128, KO, DH], BF16, kind="ExternalInput").ap()
    wtb = nc.dram_tensor("wtb", [128, KO, 4 * DH], BF16, kind="ExternalInput").ap()
    bqk_d = nc.dram_tensor("bqk", [128, 2 * HPC], F32, kind="ExternalInput").ap()
    bias_v = nc.dram_tensor("bias_v", [HPC * DH], F32, kind="ExternalInput").ap()
    out = nc.dram_tensor("out", [SC, HPC * DH], F32, kind="ExternalOutput").ap()
    with tile.TileContext(nc) as tc:
        with ExitStack() as ctx:
            _mha_body(ctx, tc, out, x, wtak, wtaq, wtb, bqk_d, bias_v)
    nc.compile()
    return nc


_NC = None


def _get_nc():
    global _NC
    if _NC is None:
        _NC = build_program()
    return _NC


def make_in_maps(x, W_qkv, b_qkv):
    import ml_dtypes

    x = np.asarray(x, dtype=np.float32)
    W = np.asarray(W_qkv, dtype=np.float32)
    b = np.asarray(b_qkv, dtype=np.float32)
    x_bf = x.astype(ml_dtypes.bfloat16)
    in_maps = []
    for c in range(N_CORES):
        bsel = c // 4
        h0 = HPC * (c % 4)
        # x^T as [ki=128, ko=8, tok]: element (p, ko, t) = x[bsel].T[ko*128+p, t]
        xT = np.ascontiguousarray(
            x_bf[bsel].T.reshape(KO, 128, SC).transpose(1, 0, 2)
        )
        # W rows for this core's heads: q_h at h*128, k_h at D+h*128, v_h at 2D+h*128
        def wrows(block, h):  # block: 0=q, 1=k, 2=v
            r0 = block * D + (h0 + h) * DH
            return W[r0:r0 + DH]

        # wtak = k0 cols, wtaq = q0 cols; wtb m-order: [k1 q1 v0 v1]
        wb = np.concatenate([wrows(1, 1), wrows(0, 1), wrows(2, 0), wrows(2, 1)], axis=0)

        def pack_wt(wm):  # [m, 1024] -> [ki=128, ko=8, m]
            wt = wm.T.astype(ml_dtypes.bfloat16)  # [1024, m]
            return np.ascontiguousarray(wt.reshape(KO, 128, wm.shape[0]).transpose(1, 0, 2))

        # bias host order: [q0 q1 k0 k1 v0 v1] blocks of 128
        brows = np.concatenate([
            b[(h0 + 0) * DH:(h0 + 1) * DH],
            b[(h0 + 1) * DH:(h0 + 2) * DH],
            b[D + (h0 + 0) * DH:D + (h0 + 1) * DH],
            b[D + (h0 + 1) * DH:D + (h0 + 2) * DH],
            b[2 * D + (h0 + 0) * DH:2 * D + (h0 + 1) * DH],
            b[2 * D + (h0 + 1) * DH:2 * D + (h0 + 2) * DH],
        ])
        in_maps.append(
            {
                "x": xT,
                "wtak": pack_wt(wrows(1, 0)),
                "wtaq": pack_wt(wrows(0, 0)),
                "wtb": pack_wt(wb),
                "bqk": np.ascontiguousarray(brows[:512].reshape(4, 128).T),
                "bias_v": np.ascontiguousarray(brows[512:]),
            }
        )
    return in_maps


def gather_output(results):
    outp = np.empty((B, S, D), np.float32)
    for c in range(N_CORES):
        o = results[c]["out"]
        bsel = c // 4
        h0 = HPC * (c % 4)
        outp[bsel, :, h0 * DH:(h0 + HPC) * DH] = o
    return outp


def kernel(x, W_qkv, b_qkv, **run_kwargs):
    in_maps = make_in_maps(x, W_qkv, b_qkv)
    res = run_bass_kernel_spmd(_get_nc(), in_maps, core_ids=list(range(N_CORES)), **run_kwargs)
    out = gather_output(res.results)
    if run_kwargs:
        kernel.last_result = res
    return out
